# revision 33
# baseline (speedup 1.0000x reference)
"""DeepseekMoE layer on 8 Trainium2 NeuronCores (Bass/Tile, expert-parallel).

Sharding (per the expert-parallel hint):
  - 16 routed experts -> 2 per core ("slot 0" = the 8 largest-load experts,
    "slot 1" = the 8 smallest, paired big+small per core). Token dispatch
    (all-to-all) is emulated at the sharding layer: the host computes the
    discrete top-4 routing, gathers each expert's tokens into a compact
    transposed batch, and scatter-adds the compact expert outputs back into
    the full output ("combine").
  - Shared expert is tensor-parallel over its intermediate dim: 2816 = 22
    tiles of 128.  16 tiles go 2-per-core; the remaining 6 are token-split
    into 4 windows of 256 tokens, 3 windows per core.  A per-core rotation
    of the token order makes the windows land at fixed offsets so the SPMD
    program is identical on every core (only DMA'd data differs).
  - Gate (softmax + renormalized top-4 combine weights) is computed ON
    DEVICE per dispatched slot directly from the gathered activations; the
    host only supplies the discrete 0/1 top-4 mask rows (routing decision).
    Gate weights and mask columns are permuted per slot so the slot's own
    expert sits in column 0.

All FLOPs that produce output values run on device.  Matmuls run in bf16
(1 cycle/row on the PE, same as f32r, but half the HBM/SBUF traffic — the
f32 version of this kernel was DMA-bound at ~88% DMA occupancy).  PSUM
accumulation is fp32.

DMA layout: a single queue sustains only ~100-200 GB/s and aggregate HBM
is ~358 GB/s/core, so the routed weight stream (the dominant traffic) is
split per m-tile across the sync and gpsimd queues, with each m's wg/wu
pair also straddling both queues.  Activation-side inputs ride the scalar
(Activation-engine) queue; the big xt / xg1 transfers are triggered from
inside the upgate(0) scalar stream so they don't steal HBM bandwidth from
the startup-critical weight stream.
"""

import math
import os
import numpy as np
import ml_dtypes

H = 2048          # hidden size
E = 16            # routed experts
TOPK = 4
I = 1408          # routed expert intermediate
ISH = 2816        # shared expert intermediate
T = 1024          # tokens
P = 128
NCORES = 8
EPC = 2           # experts per core
KH = H // P                          # 16 k-tiles over H
MI = I // P                          # 11 m-tiles over I
MH = H // P                          # 16 m-tiles over H
KI = I // P                          # 11 k-tiles over I
NFT = 2                              # full shared tiles per core
NUT = 3                              # token-split shared units per core
UW = 256                             # tokens per split window

BF16 = ml_dtypes.bfloat16

_NC_CACHE = {}
LAST_RESULTS = None  # BassKernelResults of the most recent run (for test.py)


def _chunks(C, step):
    out = []
    off = 0
    while off < C:
        sz = min(step, C - off)
        out.append((off, sz))
        off += sz
    return out


def _pack_st(w, KT, MT):
    """[KT*P, MT*P] -> [MT*P, KT*P] tile-major stationary pack.

    packed[m*P + p, k*P + c] = w[k*P + p, m*P + c], so the device loads
    rows [m*P, (m+1)*P) as one [P, KT*P] block whose column-slice k is the
    stationary tile for (k, m).
    """
    return np.ascontiguousarray(
        w.reshape(KT, P, MT, P).transpose(2, 1, 0, 3).reshape(MT * P, KT * P))


def _build(C0, C1):
    import concourse.bacc as bacc
    import concourse.mybir as mybir
    import concourse.tile as tile
    from concourse.masks import make_identity

    f32 = mybir.dt.float32
    bf16 = mybir.dt.bfloat16
    SILU = mybir.ActivationFunctionType.Silu
    EXP = mybir.ActivationFunctionType.Exp
    X = mybir.AxisListType.X

    CS = [C0, C1]
    CT = C0 + C1
    NCH = [math.ceil(c / P) for c in CS]
    XQ = 4                                   # xg0 load split (k-tile groups)
    KG = KH // XQ

    nc = bacc.Bacc("TRN2", target_bir_lowering=False, debug=False)

    xg_h = [nc.dram_tensor(f"xg{j}", [P, KH * CS[j]], bf16, kind="ExternalInput")
            for j in range(EPC)]
    mk_h = [nc.dram_tensor(f"mk{j}", [P, NCH[j] * E], f32, kind="ExternalInput")
            for j in range(EPC)]
    gwt_h = [nc.dram_tensor(f"gwt{j}", [P, KH * E], bf16, kind="ExternalInput")
             for j in range(EPC)]
    xt_h = nc.dram_tensor("xt", [P, KH * T], bf16, kind="ExternalInput")
    wg_h = [nc.dram_tensor(f"wg{j}", [I, H], bf16, kind="ExternalInput") for j in range(EPC)]
    wu_h = [nc.dram_tensor(f"wu{j}", [I, H], bf16, kind="ExternalInput") for j in range(EPC)]
    wd_h = [nc.dram_tensor(f"wd{j}", [H, I], bf16, kind="ExternalInput") for j in range(EPC)]
    swgf_h = nc.dram_tensor("swgf", [NFT * P, KH * P], bf16, kind="ExternalInput")
    swuf_h = nc.dram_tensor("swuf", [NFT * P, KH * P], bf16, kind="ExternalInput")
    swdf_h = nc.dram_tensor("swdf", [MH * P, NFT * P], bf16, kind="ExternalInput")
    swgu_h = nc.dram_tensor("swgu", [NUT * P, KH * P], bf16, kind="ExternalInput")
    swuu_h = nc.dram_tensor("swuu", [NUT * P, KH * P], bf16, kind="ExternalInput")
    sdu_h = nc.dram_tensor("sdu", [NUT * P, H], bf16, kind="ExternalInput")
    zt_h = nc.dram_tensor("zt", [H, CT], bf16, kind="ExternalOutput")
    st_h = nc.dram_tensor("st", [H, T], bf16, kind="ExternalOutput")

    with tile.TileContext(nc) as tc:
        with (
            tc.tile_pool(name="resident", bufs=1) as res_pool,
            tc.tile_pool(name="xgp", bufs=1) as xg_pool,
            tc.tile_pool(name="acts", bufs=1) as act_pool,
            tc.tile_pool(name="wst_s", bufs=4) as wst_s,
            tc.tile_pool(name="wst_g", bufs=4) as wst_g,
            tc.tile_pool(name="dst_s", bufs=3) as dst_s,
            tc.tile_pool(name="dst_g", bufs=3) as dst_g,
            tc.tile_pool(name="sstream", bufs=4) as sst_pool,
            tc.tile_pool(name="small", bufs=2) as small_pool,
            tc.tile_pool(name="stage", bufs=3) as stage_pool,
            tc.tile_pool(name="ps", bufs=1, space="PSUM") as ps_pool,
        ):
            # ---------------- resident tiles / early loads ----------------
            ident = res_pool.tile([P, P], f32, name="ident", tag="ident")
            make_identity(nc, ident[:])
            zbias = res_pool.tile([P, 1], f32, name="zbias", tag="zbias")
            nc.vector.memset(zbias[:], 0.0)

            # scalar queue, front: only what the first routed phase needs.
            # xg0 loads in k-tile groups (separate tiles) so the very first
            # upgate matmuls start as soon as group 0 lands.
            xg0q = [xg_pool.tile([P, KG * CS[0]], bf16, name=f"xg0q{q}", tag=f"xg0q{q}")
                    for q in range(XQ)]
            for q in range(XQ):
                nc.scalar.dma_start(
                    xg0q[q][:], xg_h[0][:, q * KG * CS[0]:(q + 1) * KG * CS[0]])
            gwtb = [res_pool.tile([P, KH * E], bf16, name=f"gwtb{j}", tag=f"gwtb{j}")
                    for j in range(EPC)]
            nc.scalar.dma_start(gwtb[0][:], gwt_h[0][:])
            nc.scalar.dma_start(gwtb[1][:], gwt_h[1][:])
            mkb = [res_pool.tile([P, NCH[j] * E], f32, name=f"mkb{j}", tag=f"mkb{j}")
                   for j in range(EPC)]
            nc.scalar.dma_start(mkb[0][:], mk_h[0][:])
            nc.scalar.dma_start(mkb[1][:], mk_h[1][:])
            # xt / xg1 / unit-down weights are triggered from inside
            # upgate(0)'s scalar stream (deferred, see scalar_hooks below)
            xgb1 = xg_pool.tile([P, KH * CS[1]], bf16, name="xgb1", tag="xgb1")
            xtb = res_pool.tile([P, KH * T], bf16, name="xtb", tag="xtb")
            xt_t = [xtb[:, k * T:(k + 1) * T] for k in range(KH)]
            sdub = [res_pool.tile([P, H], bf16, name=f"sdub{i}", tag=f"sdub{i}")
                    for i in range(NUT)]

            def defer_loads(m):
                if m == 1:
                    nc.scalar.dma_start(xgb1[:], xg_h[1][:])
                if m in (5, 7, 9):
                    q = (5, 7, 9).index(m)
                    nc.scalar.dma_start(xtb[:, q * 4 * T:(q + 1) * 4 * T],
                                        xt_h[:, q * 4 * T:(q + 1) * 4 * T])

            # per-slot xg k-tile views
            xg_sl = [
                [xg0q[k // KG][:, (k % KG) * CS[0]:(k % KG + 1) * CS[0]] for k in range(KH)],
                [xgb1[:, k * CS[1]:(k + 1) * CS[1]] for k in range(KH)],
            ]

            wb = [res_pool.tile([P, CS[j]], f32, name=f"wb{j}", tag=f"wb{j}")
                  for j in range(EPC)]
            wcol = [[res_pool.tile([P, 1], f32, name=f"wcol{j}_{ch}", tag=f"wcol{j}_{ch}")
                     for ch in range(NCH[j])] for j in range(EPC)]
            a_t = [[act_pool.tile([P, CS[j]], bf16, name=f"a{j}_{m}", tag=f"a{j}_{m}")
                    for m in range(MI)] for j in range(EPC)]
            sg_t = [act_pool.tile([P, T], bf16, name=f"sg{m}", tag="sgtmp", bufs=2)
                    for m in range(NFT)]
            as_t = [act_pool.tile([P, T], bf16, name=f"as{m}", tag=f"as{m}")
                    for m in range(NFT)]
            asu_t = [act_pool.tile([P, UW], bf16, name=f"asu{i}", tag=f"asu{i}")
                     for i in range(NUT)]

            # ---------------- emission sections ----------------
            def emit_gate_logits(j):
                # per-slot combine weights straight from the gathered
                # activations: logits with tokens stationary -> [csz, E]
                # token-major (no transpose); own expert = column 0.
                C = CS[j]
                lg = ps_pool.tile([P, NCH[j] * E], f32, name=f"lg{j}", tag="A1", bufs=4)
                for ch in range(NCH[j]):
                    coff = ch * P
                    csz = min(P, C - coff)
                    for k in range(KH):
                        nc.tensor.matmul(
                            lg[:csz, ch * E:(ch + 1) * E],
                            lhsT=xg_sl[j][k][:, coff:coff + csz],
                            rhs=gwtb[j][:, k * E:(k + 1) * E],
                            start=(k == 0), stop=(k == KH - 1),
                        )
                for ch in range(NCH[j]):
                    coff = ch * P
                    csz = min(P, C - coff)
                    sc = small_pool.tile([P, E], f32, name=f"sc{j}_{ch}", tag="sc")
                    nc.scalar.activation(sc[:csz], lg[:csz, ch * E:(ch + 1) * E],
                                         EXP, bias=zbias[:csz])
                    mskd = small_pool.tile([P, E], f32, name=f"mskd{j}_{ch}", tag="mskd")
                    nc.vector.tensor_mul(out=mskd[:csz], in0=sc[:csz],
                                         in1=mkb[j][:csz, ch * E:(ch + 1) * E])
                    ssum = small_pool.tile([P, 1], f32, name=f"ssum{j}_{ch}", tag="ssum")
                    nc.vector.reduce_sum(ssum[:csz], mskd[:csz], axis=X)
                    rsum = small_pool.tile([P, 1], f32, name=f"rsum{j}_{ch}", tag="rsum")
                    nc.vector.reciprocal(rsum[:csz], ssum[:csz])
                    nc.vector.tensor_scalar_mul(wcol[j][ch][:csz], mskd[:csz, 0:1],
                                                rsum[:csz, :1])

            def emit_gate_broadcast(j):
                # partition-broadcast of the per-slot weights: emitted well
                # after the logits so the PE never waits on the DVE chain.
                C = CS[j]
                for ch in range(NCH[j]):
                    coff = ch * P
                    csz = min(P, C - coff)
                    wbps = ps_pool.tile([P, P], f32, name=f"wbps{j}_{ch}", tag="A1", bufs=4)
                    nc.tensor.transpose(
                        out=wbps[:, :csz],
                        in_=wcol[j][ch][:csz, :1].to_broadcast([csz, P]),
                        identity=ident[0:csz, 0:csz],
                    )
                    nc.vector.tensor_copy(wb[j][:, coff:coff + csz], wbps[:, :csz])

            def emit_upgate(j, hooks=None):
                C = CS[j]
                xg_t = xg_sl[j]
                g_t = [act_pool.tile([P, C], bf16, name=f"g{j}_{m}", tag="gtmp", bufs=3)
                       for m in range(MI)]
                for m in range(MI):
                    # wg/wu of each m ride different queues (halves the
                    # per-queue burst for the tile the PE is waiting on)
                    pa, ea, pb, eb = ((wst_s, nc.sync, wst_g, nc.gpsimd) if m % 2 == 0
                                      else (wst_g, nc.gpsimd, wst_s, nc.sync))
                    if j == 0 and m == 0:
                        # cold start: split the first pair's halves across
                        # both queues so the head tile lands ~2x sooner
                        wga = wst_s.tile([P, 8 * P], bf16, name="wg00a", tag="wblk", bufs=4)
                        nc.sync.dma_start(wga[:], wg_h[0][0:P, :8 * P])
                        wgb_ = wst_g.tile([P, 8 * P], bf16, name="wg00b", tag="wblk", bufs=4)
                        nc.gpsimd.dma_start(wgb_[:], wg_h[0][0:P, 8 * P:])
                        wua = wst_s.tile([P, 8 * P], bf16, name="wu00a", tag="wblk", bufs=4)
                        nc.sync.dma_start(wua[:], wu_h[0][0:P, :8 * P])
                        wub_ = wst_g.tile([P, 8 * P], bf16, name="wu00b", tag="wblk", bufs=4)
                        nc.gpsimd.dma_start(wub_[:], wu_h[0][0:P, 8 * P:])
                        wgk = lambda k: (wga if k < 8 else wgb_)[:, (k % 8) * P:(k % 8 + 1) * P]
                        wuk = lambda k: (wua if k < 8 else wub_)[:, (k % 8) * P:(k % 8 + 1) * P]
                    else:
                        wgb = pa.tile([P, KH * P], bf16, name=f"wgb{j}_{m}", tag="wblk", bufs=4)
                        ea.dma_start(wgb[:], wg_h[j][m * P:(m + 1) * P, :])
                        wub = pb.tile([P, KH * P], bf16, name=f"wub{j}_{m}", tag="wblk", bufs=4)
                        eb.dma_start(wub[:], wu_h[j][m * P:(m + 1) * P, :])
                        wgk = lambda k: wgb[:, k * P:(k + 1) * P]
                        wuk = lambda k: wub[:, k * P:(k + 1) * P]
                    for (coff, csz) in _chunks(C, 512):
                        psg = ps_pool.tile([P, csz], f32, name=f"psg{j}_{m}_{coff}", tag="A1", bufs=4)
                        for k in range(KH):
                            nc.tensor.matmul(psg[:], lhsT=wgk(k),
                                             rhs=xg_t[k][:, coff:coff + csz],
                                             start=(k == 0), stop=(k == KH - 1))
                        nc.scalar.activation(g_t[m][:, coff:coff + csz], psg[:], SILU, bias=zbias[:])
                        psu = ps_pool.tile([P, csz], f32, name=f"psu{j}_{m}_{coff}", tag="A1", bufs=4)
                        for k in range(KH):
                            nc.tensor.matmul(psu[:], lhsT=wuk(k),
                                             rhs=xg_t[k][:, coff:coff + csz],
                                             start=(k == 0), stop=(k == KH - 1))
                        # a = silu(g) * u straight out of PSUM, rounded to bf16
                        nc.vector.tensor_mul(out=a_t[j][m][:, coff:coff + csz],
                                             in0=g_t[m][:, coff:coff + csz], in1=psu[:])
                    if hooks is not None:
                        hooks(m)

            def emit_down(j, hooks=None):
                C = CS[j]
                off_j = 0 if j == 0 else C0
                for m in range(MH):
                    if hooks is not None:
                        hooks(m)
                    pool, eng = (dst_s, nc.sync) if m % 2 == 0 else (dst_g, nc.gpsimd)
                    wdb = pool.tile([P, KI * P], bf16, name=f"wdb{j}_{m}", tag="wdb", bufs=3)
                    eng.dma_start(wdb[:], wd_h[j][m * P:(m + 1) * P, :])
                    for (coff, csz) in _chunks(C, 512):
                        psz = ps_pool.tile([P, csz], f32, name=f"psz{j}_{m}_{coff}", tag="A1", bufs=4)
                        for k in range(KI):
                            nc.tensor.matmul(psz[:], lhsT=wdb[:, k * P:(k + 1) * P],
                                             rhs=a_t[j][k][:, coff:coff + csz],
                                             start=(k == 0), stop=(k == KI - 1))
                        zst = stage_pool.tile([P, csz], bf16, name=f"zst{j}_{m}_{coff}", tag="zst", bufs=3)
                        # combine-weight scaling fused into the eviction.
                        # zt rides the scalar queue: a data-dependent trigger
                        # on a weight queue would head-of-line block the
                        # weight stream behind it.
                        nc.vector.tensor_mul(out=zst[:], in0=wb[j][:, coff:coff + csz], in1=psz[:])
                        nc.scalar.dma_start(
                            zt_h[m * P:(m + 1) * P, off_j + coff:off_j + coff + csz], zst[:])

            # full-shared weights: loaded early on the scalar queue (both
            # weight queues are backlogged with routed streams when the
            # shared phases start — sub0 arrived ~10us late on sync)
            sgbt = [None] * NFT
            subt = [None] * NFT

            def load_shared_full(mi):
                sgbt[mi] = sst_pool.tile([P, KH * P], bf16, name=f"sgb{mi}", tag="ssb", bufs=4)
                nc.scalar.dma_start(sgbt[mi][:], swgf_h[mi * P:(mi + 1) * P, :])
                subt[mi] = sst_pool.tile([P, KH * P], bf16, name=f"sub{mi}", tag="ssb", bufs=4)
                nc.scalar.dma_start(subt[mi][:], swuf_h[mi * P:(mi + 1) * P, :])

            def emit_shared_ug_full(mi):
                sgb = sgbt[mi]
                psgs = ps_pool.tile([P, T], f32, name=f"psgs{mi}", tag="B1", bufs=2)
                for k in range(KH):
                    for (noff, nsz) in _chunks(T, 512):
                        nc.tensor.matmul(psgs[:, noff:noff + nsz],
                                         lhsT=sgb[:, k * P:(k + 1) * P],
                                         rhs=xt_t[k][:, noff:noff + nsz],
                                         start=(k == 0), stop=(k == KH - 1))
                nc.scalar.activation(sg_t[mi][:], psgs[:], SILU, bias=zbias[:])
                sub = subt[mi]
                psus = ps_pool.tile([P, T], f32, name=f"psus{mi}", tag="B1", bufs=2)
                for k in range(KH):
                    for (noff, nsz) in _chunks(T, 512):
                        nc.tensor.matmul(psus[:, noff:noff + nsz],
                                         lhsT=sub[:, k * P:(k + 1) * P],
                                         rhs=xt_t[k][:, noff:noff + nsz],
                                         start=(k == 0), stop=(k == KH - 1))
                nc.vector.tensor_mul(out=as_t[mi][:], in0=sg_t[mi][:], in1=psus[:])

            def emit_shared_ug_unit(i):
                w0 = i * UW
                sgu = sst_pool.tile([P, KH * P], bf16, name=f"sgu{i}", tag="ssb", bufs=4)
                nc.gpsimd.dma_start(sgu[:], swgu_h[i * P:(i + 1) * P, :])
                psg = ps_pool.tile([P, UW], f32, name=f"psgu{i}", tag="A1", bufs=4)
                for k in range(KH):
                    nc.tensor.matmul(psg[:], lhsT=sgu[:, k * P:(k + 1) * P],
                                     rhs=xt_t[k][:, w0:w0 + UW],
                                     start=(k == 0), stop=(k == KH - 1))
                sgt = small_pool.tile([P, UW], bf16, name=f"sgut{i}", tag="sgut")
                nc.scalar.activation(sgt[:], psg[:], SILU, bias=zbias[:])
                suu = sst_pool.tile([P, KH * P], bf16, name=f"suu{i}", tag="ssb", bufs=4)
                nc.sync.dma_start(suu[:], swuu_h[i * P:(i + 1) * P, :])
                psu = ps_pool.tile([P, UW], f32, name=f"psuu{i}", tag="A1", bufs=4)
                for k in range(KH):
                    nc.tensor.matmul(psu[:], lhsT=suu[:, k * P:(k + 1) * P],
                                     rhs=xt_t[k][:, w0:w0 + UW],
                                     start=(k == 0), stop=(k == KH - 1))
                nc.vector.tensor_mul(out=asu_t[i][:], in0=sgt[:], in1=psu[:])

            def emit_shared_down():
                for m in range(MH):
                    # dedicated pool + alternating queues: sharing the ssb
                    # pool made sdb loads wait on unit-weight slot frees
                    pool, eng = (dst_s, nc.sync) if m % 2 == 0 else (dst_g, nc.gpsimd)
                    sdb = pool.tile([P, NFT * P], bf16, name=f"sdb{m}", tag="sdb", bufs=6)
                    eng.dma_start(sdb[:], swdf_h[m * P:(m + 1) * P, :])
                    psys = ps_pool.tile([P, T], f32, name=f"psys{m}", tag="B1", bufs=2)
                    for r in range(4):
                        r0 = r * UW
                        nc.tensor.matmul(psys[:, r0:r0 + UW], lhsT=sdb[:, 0:P],
                                         rhs=as_t[0][:, r0:r0 + UW],
                                         start=True, stop=False)
                        nc.tensor.matmul(psys[:, r0:r0 + UW], lhsT=sdb[:, P:2 * P],
                                         rhs=as_t[1][:, r0:r0 + UW],
                                         start=False, stop=(r == 3))
                        if r < 3:
                            nc.tensor.matmul(psys[:, r0:r0 + UW],
                                             lhsT=sdub[r][:, m * P:(m + 1) * P],
                                             rhs=asu_t[r][:],
                                             start=False, stop=True)
                    sstg = stage_pool.tile([P, T], bf16, name=f"sstg{m}", tag="sstage", bufs=3)
                    # evict halves on vector+scalar in parallel: a single
                    # engine's f32->bf16 cast (~1.2us per [128,1024]) paced
                    # the whole tail through the 2-slot PSUM pool
                    nc.vector.tensor_copy(sstg[:, 0:T // 2], psys[:, 0:T // 2])
                    nc.scalar.copy(sstg[:, T // 2:T], psys[:, T // 2:T])
                    # st writes round-robin three queues (a single queue's
                    # trailing writes paced the end-of-kernel drain)
                    eng = (nc.scalar, nc.gpsimd, nc.sync)[m % 3]
                    eng.dma_start(st_h[m * P:(m + 1) * P, :], sstg[:])

            def up0_hooks(m):
                defer_loads(m)
                # gate logits after the last silu: a mid-phase EXP would
                # thrash the scalar engine's activation tables (1.3us per
                # reload) right where the a_t chain is critical
                if m == MI - 1:
                    load_shared_full(0)
                    emit_gate_logits(0)
                    emit_gate_logits(1)

            def down0_hooks(m):
                # xt tail + unit-down weights land here (the shared_ug(0)
                # window has DMA slack; triggering them during upgate(0)
                # oversubscribed the down(0) entry)
                if m == 2:
                    nc.scalar.dma_start(xtb[:, 12 * T:16 * T], xt_h[:, 12 * T:16 * T])
                if m == 4:
                    for i in range(NUT):
                        nc.scalar.dma_start(sdub[i][:], sdu_h[i * P:(i + 1) * P, :])
                if m == 8:
                    load_shared_full(1)

            emit_upgate(0, hooks=up0_hooks)
            emit_gate_broadcast(0)
            emit_gate_broadcast(1)
            emit_down(0, hooks=down0_hooks)
            emit_shared_ug_full(0)
            emit_upgate(1)
            emit_down(1)
            emit_shared_ug_full(1)
            emit_shared_ug_unit(0)
            emit_shared_ug_unit(1)
            emit_shared_ug_unit(2)
            emit_shared_down()

    nc.compile()
    return nc


def _get_nc(C0, C1):
    key = (C0, C1)
    if key not in _NC_CACHE:
        _NC_CACHE[key] = _build(C0, C1)
    return _NC_CACHE[key]


def kernel(**inputs):
    global LAST_RESULTS
    from concourse.bass_utils import run_bass_kernel_spmd

    hs = np.asarray(inputs["hidden_states"], dtype=np.float32)
    gate_w = np.asarray(inputs["gate_w"], dtype=np.float32)
    w_gate = np.asarray(inputs["w_gate"], dtype=np.float32)
    w_up = np.asarray(inputs["w_up"], dtype=np.float32)
    w_down = np.asarray(inputs["w_down"], dtype=np.float32)
    sw_gate = np.asarray(inputs["sw_gate"], dtype=np.float32)
    sw_up = np.asarray(inputs["sw_up"], dtype=np.float32)
    sw_down = np.asarray(inputs["sw_down"], dtype=np.float32)

    orig_shape = hs.shape
    x = hs.reshape(-1, H)
    assert x.shape[0] == T

    # ---- host: discrete routing only (top-4 selection + dispatch tables) ----
    logits = x @ gate_w.T
    smax = logits.max(axis=-1, keepdims=True)
    sc = np.exp(logits - smax)
    sc /= sc.sum(axis=-1, keepdims=True)
    order = np.argsort(-sc, axis=-1, kind="stable")[:, :TOPK]
    mask = np.zeros((T, E), dtype=np.float32)
    mask[np.arange(T)[:, None], order] = 1.0
    tok_lists = [np.nonzero(mask[:, e])[0].astype(np.int64) for e in range(E)]
    counts = np.array([len(tk) for tk in tok_lists])

    # slot 0 = the 8 largest-load experts, slot 1 = the 8 smallest
    rank = np.argsort(-counts, kind="stable")
    slot_experts = [rank[:NCORES], rank[NCORES:][::-1]]  # pair big with small
    C0 = int(counts[slot_experts[0]].max())
    C1 = int(counts[slot_experts[1]].max())

    nc = _get_nc(C0, C1)
    CS = [C0, C1]
    NCH = [math.ceil(c / P) for c in CS]

    xb = x.astype(BF16)
    xTb = np.ascontiguousarray(xb.T)                     # [H, T] bf16
    gate_wT = gate_w.T.astype(BF16)                      # [H, E] bf16

    swg_b = sw_gate.astype(BF16)
    swu_b = sw_up.astype(BF16)
    swd_b = sw_down.astype(BF16)
    wg_b = w_gate.astype(BF16)
    wu_b = w_up.astype(BF16)
    wd_b = w_down.astype(BF16)

    in_maps = []
    core_experts = []
    rhos = []
    for c in range(NCORES):
        es = [int(slot_experts[j][c]) for j in range(EPC)]
        core_experts.append(es)
        rho = UW * ((NUT * c) % 4)
        rhos.append(rho)
        # token rotation for the split shared tiles (windows land at fixed
        # rotated offsets 0/256/512 on every core)
        xrotT = np.roll(xTb, -rho, axis=1)               # [H, T] rotated cols
        xtb = np.ascontiguousarray(
            xrotT.reshape(KH, P, T).transpose(1, 0, 2).reshape(P, KH * T))
        # full shared tiles: intermediate cols [256c, 256c+256)
        fcols = slice(NFT * P * c, NFT * P * (c + 1))
        # split units
        ucols = []
        for i in range(NUT):
            u = NUT * c + i
            t0 = 2 * NCORES * P + (u // 4) * P           # 2048 + 128*(u//4)
            ucols.append(np.arange(t0, t0 + P))
        ucat = np.concatenate(ucols)
        im = {
            "xt": xtb,
            "swgf": _pack_st(swg_b[:, fcols], KH, NFT),
            "swuf": _pack_st(swu_b[:, fcols], KH, NFT),
            "swdf": _pack_st(swd_b[fcols, :], NFT, MH),
            "swgu": np.concatenate(
                [_pack_st(swg_b[:, uc], KH, 1) for uc in ucols], axis=0),
            "swuu": np.concatenate(
                [_pack_st(swu_b[:, uc], KH, 1) for uc in ucols], axis=0),
            "sdu": np.ascontiguousarray(swd_b[ucat, :]),
        }
        for j, e in enumerate(es):
            tk = tok_lists[e]
            n = len(tk)
            C = CS[j]
            # gathered activations, tile-major: xg[p, k*C + s] = x[tok_s, k*P + p]
            xg = np.zeros((P, KH * C), dtype=BF16)
            g = xTb[:, tk].reshape(KH, P, n).transpose(1, 0, 2)  # [P, KH, n]
            xg.reshape(P, KH, C)[:, :, :n] = g
            im[f"xg{j}"] = xg
            # gate weights + mask rows share a per-slot permutation with the
            # slot's own expert in column 0 (softmax sums are perm-invariant)
            perm = [e] + [q for q in range(E) if q != e]
            gwtp = gate_wT[:, perm]                              # [H, E]
            im[f"gwt{j}"] = np.ascontiguousarray(
                gwtp.reshape(KH, P, E).transpose(1, 0, 2).reshape(P, KH * E))
            mkc = np.zeros((P, NCH[j] * E), dtype=np.float32)
            mrows = mask[tk][:, perm]                            # [n, E]
            for ch in range(NCH[j]):
                lo = ch * P
                sz = min(P, n - lo)
                if sz > 0:
                    mkc[:sz, ch * E:(ch + 1) * E] = mrows[lo:lo + sz]
            im[f"mk{j}"] = mkc
            im[f"wg{j}"] = _pack_st(wg_b[e], KH, MI)
            im[f"wu{j}"] = _pack_st(wu_b[e], KH, MI)
            im[f"wd{j}"] = _pack_st(wd_b[e], KI, MH)
        in_maps.append(im)

    trace = bool(int(os.environ.get("BASSMOE_TRACE", "0")))
    kwargs = {}
    if trace:
        kwargs = dict(trace=True, tmpdir=os.environ.get("BASSMOE_TRACE_DIR") or None)
        tcores = os.environ.get("BASSMOE_TRACE_CORES")
        if tcores:
            kwargs["trace_cores"] = [int(x) for x in tcores.split(",")]
            kwargs["stitch_traces"] = False
    res = run_bass_kernel_spmd(nc, in_maps, core_ids=list(range(NCORES)), **kwargs)
    LAST_RESULTS = res

    # ---- host: unshard (scatter-add compact expert outputs + sum partials) ----
    y = np.zeros((T, H), dtype=np.float64)
    st_sum = np.zeros((H, T), dtype=np.float64)
    for c in range(NCORES):
        r = res.results[c]
        # st is in this core's rotated token order; unrotate
        st_sum += np.roll(np.asarray(r["st"], dtype=np.float32), rhos[c], axis=1)
        zt = np.asarray(r["zt"], dtype=np.float32)
        for j in range(EPC):
            e = core_experts[c][j]
            tk = tok_lists[e]
            off = 0 if j == 0 else C0
            y[tk] += zt[:, off:off + len(tk)].T
    y += st_sum.T
    return y.astype(np.float32).reshape(orig_shape)


# revision 34
# speedup vs baseline: 1.0533x; 1.0533x over previous
"""DeepseekMoE layer on 8 Trainium2 NeuronCores (Bass/Tile, expert-parallel).

Sharding (per the expert-parallel hint):
  - 16 routed experts -> 2 per core ("slot 0" = the 8 largest-load experts,
    "slot 1" = the 8 smallest, paired big+small per core). Token dispatch
    (all-to-all) is emulated at the sharding layer: the host computes the
    discrete top-4 routing, gathers each expert's tokens into a compact
    transposed batch, and scatter-adds the compact expert outputs back into
    the full output ("combine").
  - Shared expert is tensor-parallel over its intermediate dim: 2816 = 22
    tiles of 128.  16 tiles go 2-per-core; the remaining 6 are token-split
    into 4 windows of 256 tokens, 3 windows per core.  A per-core rotation
    of the token order makes the windows land at fixed offsets so the SPMD
    program is identical on every core (only DMA'd data differs).
  - Gate (softmax + renormalized top-4 combine weights) is computed ON
    DEVICE per dispatched slot directly from the gathered activations; the
    host only supplies the discrete 0/1 top-4 mask rows (routing decision).
    Gate weights and mask columns are permuted per slot so the slot's own
    expert sits in column 0.

All FLOPs that produce output values run on device.  Matmuls run in bf16
(1 cycle/row on the PE, same as f32r, but half the HBM/SBUF traffic — the
f32 version of this kernel was DMA-bound at ~88% DMA occupancy).  PSUM
accumulation is fp32.

DMA layout: a single queue sustains only ~100-200 GB/s and aggregate HBM
is ~358 GB/s/core, so the routed weight stream (the dominant traffic) is
split per m-tile across the sync and gpsimd queues, with each m's wg/wu
pair also straddling both queues.  Activation-side inputs ride the scalar
(Activation-engine) queue; the big xt / xg1 transfers are triggered from
inside the upgate(0) scalar stream so they don't steal HBM bandwidth from
the startup-critical weight stream.
"""

import math
import os
import numpy as np
import ml_dtypes

H = 2048          # hidden size
E = 16            # routed experts
TOPK = 4
I = 1408          # routed expert intermediate
ISH = 2816        # shared expert intermediate
T = 1024          # tokens
P = 128
NCORES = 8
EPC = 2           # experts per core
KH = H // P                          # 16 k-tiles over H
MI = I // P                          # 11 m-tiles over I
MH = H // P                          # 16 m-tiles over H
KI = I // P                          # 11 k-tiles over I
NFT = 2                              # full shared tiles per core
NUT = 3                              # token-split shared units per core
UW = 256                             # tokens per split window

BF16 = ml_dtypes.bfloat16

_NC_CACHE = {}
LAST_RESULTS = None  # BassKernelResults of the most recent run (for test.py)


def _chunks(C, step):
    out = []
    off = 0
    while off < C:
        sz = min(step, C - off)
        out.append((off, sz))
        off += sz
    return out


def _pack_st(w, KT, MT):
    """[KT*P, MT*P] -> [MT*P, KT*P] tile-major stationary pack.

    packed[m*P + p, k*P + c] = w[k*P + p, m*P + c], so the device loads
    rows [m*P, (m+1)*P) as one [P, KT*P] block whose column-slice k is the
    stationary tile for (k, m).
    """
    return np.ascontiguousarray(
        w.reshape(KT, P, MT, P).transpose(2, 1, 0, 3).reshape(MT * P, KT * P))


def _build(C0, C1):
    import concourse.bacc as bacc
    import concourse.mybir as mybir
    import concourse.tile as tile
    from concourse.masks import make_identity

    f32 = mybir.dt.float32
    bf16 = mybir.dt.bfloat16
    SILU = mybir.ActivationFunctionType.Silu
    EXP = mybir.ActivationFunctionType.Exp
    X = mybir.AxisListType.X

    CS = [C0, C1]
    CT = C0 + C1
    NCH = [math.ceil(c / P) for c in CS]
    XQ = 4                                   # xg0 load split (k-tile groups)
    KG = KH // XQ

    nc = bacc.Bacc("TRN2", target_bir_lowering=False, debug=False)

    xg_h = [nc.dram_tensor(f"xg{j}", [P, KH * CS[j]], bf16, kind="ExternalInput")
            for j in range(EPC)]
    mk_h = [nc.dram_tensor(f"mk{j}", [P, NCH[j] * E], f32, kind="ExternalInput")
            for j in range(EPC)]
    gwt_h = [nc.dram_tensor(f"gwt{j}", [P, KH * E], bf16, kind="ExternalInput")
             for j in range(EPC)]
    xt_h = nc.dram_tensor("xt", [P, KH * T], bf16, kind="ExternalInput")
    wg_h = [nc.dram_tensor(f"wg{j}", [I, H], bf16, kind="ExternalInput") for j in range(EPC)]
    wu_h = [nc.dram_tensor(f"wu{j}", [I, H], bf16, kind="ExternalInput") for j in range(EPC)]
    wd_h = [nc.dram_tensor(f"wd{j}", [H, I], bf16, kind="ExternalInput") for j in range(EPC)]
    swgf_h = nc.dram_tensor("swgf", [NFT * P, KH * P], bf16, kind="ExternalInput")
    swuf_h = nc.dram_tensor("swuf", [NFT * P, KH * P], bf16, kind="ExternalInput")
    swdf_h = nc.dram_tensor("swdf", [MH * P, NFT * P], bf16, kind="ExternalInput")
    swgu_h = nc.dram_tensor("swgu", [NUT * P, KH * P], bf16, kind="ExternalInput")
    swuu_h = nc.dram_tensor("swuu", [NUT * P, KH * P], bf16, kind="ExternalInput")
    sdu_h = nc.dram_tensor("sdu", [NUT * P, H], bf16, kind="ExternalInput")
    zt_h = nc.dram_tensor("zt", [H, CT], bf16, kind="ExternalOutput")
    st_h = nc.dram_tensor("st", [H, T], bf16, kind="ExternalOutput")

    with tile.TileContext(nc) as tc:
        with (
            tc.tile_pool(name="resident", bufs=1) as res_pool,
            tc.tile_pool(name="xgp", bufs=1) as xg_pool,
            tc.tile_pool(name="acts", bufs=1) as act_pool,
            tc.tile_pool(name="wst_s", bufs=4) as wst_s,
            tc.tile_pool(name="wst_g", bufs=4) as wst_g,
            tc.tile_pool(name="dst_s", bufs=3) as dst_s,
            tc.tile_pool(name="dst_g", bufs=3) as dst_g,
            tc.tile_pool(name="sstream", bufs=4) as sst_pool,
            tc.tile_pool(name="small", bufs=2) as small_pool,
            tc.tile_pool(name="stage", bufs=3) as stage_pool,
            tc.tile_pool(name="ps", bufs=1, space="PSUM") as ps_pool,
        ):
            # ---------------- resident tiles / early loads ----------------
            ident = res_pool.tile([P, P], f32, name="ident", tag="ident")
            make_identity(nc, ident[:])
            zbias = res_pool.tile([P, 1], f32, name="zbias", tag="zbias")
            nc.vector.memset(zbias[:], 0.0)

            # scalar queue, front: only what the first routed phase needs.
            # xg0 loads in k-tile groups (separate tiles) so the very first
            # upgate matmuls start as soon as group 0 lands.
            xg0q = [xg_pool.tile([P, KG * CS[0]], bf16, name=f"xg0q{q}", tag=f"xg0q{q}")
                    for q in range(XQ)]
            for q in range(XQ):
                nc.scalar.dma_start(
                    xg0q[q][:], xg_h[0][:, q * KG * CS[0]:(q + 1) * KG * CS[0]])
            gwtb = [res_pool.tile([P, KH * E], bf16, name=f"gwtb{j}", tag=f"gwtb{j}")
                    for j in range(EPC)]
            nc.scalar.dma_start(gwtb[0][:], gwt_h[0][:])
            nc.scalar.dma_start(gwtb[1][:], gwt_h[1][:])
            mkb = [res_pool.tile([P, NCH[j] * E], f32, name=f"mkb{j}", tag=f"mkb{j}")
                   for j in range(EPC)]
            nc.scalar.dma_start(mkb[0][:], mk_h[0][:])
            nc.scalar.dma_start(mkb[1][:], mk_h[1][:])
            # xt / xg1 / unit-down weights are triggered from inside
            # upgate(0)'s scalar stream (deferred, see scalar_hooks below)
            xgb1 = xg_pool.tile([P, KH * CS[1]], bf16, name="xgb1", tag="xgb1")
            xtb = res_pool.tile([P, KH * T], bf16, name="xtb", tag="xtb")
            xt_t = [xtb[:, k * T:(k + 1) * T] for k in range(KH)]
            sdub = [res_pool.tile([P, H], bf16, name=f"sdub{i}", tag=f"sdub{i}")
                    for i in range(NUT)]

            def defer_loads(m):
                if m == 1:
                    nc.scalar.dma_start(xgb1[:], xg_h[1][:])
                if m in (5, 7, 9, 10):
                    q = (5, 7, 9, 10).index(m)
                    nc.scalar.dma_start(xtb[:, q * 4 * T:(q + 1) * 4 * T],
                                        xt_h[:, q * 4 * T:(q + 1) * 4 * T])
                if m == 10:
                    for i in range(NUT):
                        nc.scalar.dma_start(sdub[i][:], sdu_h[i * P:(i + 1) * P, :])

            # per-slot xg k-tile views
            xg_sl = [
                [xg0q[k // KG][:, (k % KG) * CS[0]:(k % KG + 1) * CS[0]] for k in range(KH)],
                [xgb1[:, k * CS[1]:(k + 1) * CS[1]] for k in range(KH)],
            ]

            wb = [res_pool.tile([P, CS[j]], f32, name=f"wb{j}", tag=f"wb{j}")
                  for j in range(EPC)]
            wcol = [[res_pool.tile([P, 1], f32, name=f"wcol{j}_{ch}", tag=f"wcol{j}_{ch}")
                     for ch in range(NCH[j])] for j in range(EPC)]
            a_t = [[act_pool.tile([P, CS[j]], bf16, name=f"a{j}_{m}", tag=f"a{j}_{m}")
                    for m in range(MI)] for j in range(EPC)]
            sg_t = [act_pool.tile([P, T], bf16, name=f"sg{m}", tag="sgtmp", bufs=2)
                    for m in range(NFT)]
            as_t = [act_pool.tile([P, T], bf16, name=f"as{m}", tag=f"as{m}")
                    for m in range(NFT)]
            asu_t = [act_pool.tile([P, UW], bf16, name=f"asu{i}", tag=f"asu{i}")
                     for i in range(NUT)]

            # ---------------- emission sections ----------------
            def emit_gate_logits(j):
                # per-slot combine weights straight from the gathered
                # activations: logits with tokens stationary -> [csz, E]
                # token-major (no transpose); own expert = column 0.
                C = CS[j]
                lg = ps_pool.tile([P, NCH[j] * E], f32, name=f"lg{j}", tag="A1", bufs=4)
                for ch in range(NCH[j]):
                    coff = ch * P
                    csz = min(P, C - coff)
                    for k in range(KH):
                        nc.tensor.matmul(
                            lg[:csz, ch * E:(ch + 1) * E],
                            lhsT=xg_sl[j][k][:, coff:coff + csz],
                            rhs=gwtb[j][:, k * E:(k + 1) * E],
                            start=(k == 0), stop=(k == KH - 1),
                        )
                for ch in range(NCH[j]):
                    coff = ch * P
                    csz = min(P, C - coff)
                    sc = small_pool.tile([P, E], f32, name=f"sc{j}_{ch}", tag="sc")
                    nc.scalar.activation(sc[:csz], lg[:csz, ch * E:(ch + 1) * E],
                                         EXP, bias=zbias[:csz])
                    mskd = small_pool.tile([P, E], f32, name=f"mskd{j}_{ch}", tag="mskd")
                    nc.vector.tensor_mul(out=mskd[:csz], in0=sc[:csz],
                                         in1=mkb[j][:csz, ch * E:(ch + 1) * E])
                    ssum = small_pool.tile([P, 1], f32, name=f"ssum{j}_{ch}", tag="ssum")
                    nc.vector.reduce_sum(ssum[:csz], mskd[:csz], axis=X)
                    rsum = small_pool.tile([P, 1], f32, name=f"rsum{j}_{ch}", tag="rsum")
                    nc.vector.reciprocal(rsum[:csz], ssum[:csz])
                    nc.vector.tensor_scalar_mul(wcol[j][ch][:csz], mskd[:csz, 0:1],
                                                rsum[:csz, :1])

            def emit_gate_broadcast(j):
                # partition-broadcast of the per-slot weights: emitted well
                # after the logits so the PE never waits on the DVE chain.
                C = CS[j]
                for ch in range(NCH[j]):
                    coff = ch * P
                    csz = min(P, C - coff)
                    wbps = ps_pool.tile([P, P], f32, name=f"wbps{j}_{ch}", tag="A1", bufs=4)
                    nc.tensor.transpose(
                        out=wbps[:, :csz],
                        in_=wcol[j][ch][:csz, :1].to_broadcast([csz, P]),
                        identity=ident[0:csz, 0:csz],
                    )
                    nc.vector.tensor_copy(wb[j][:, coff:coff + csz], wbps[:, :csz])

            def emit_upgate(j, hooks=None):
                C = CS[j]
                xg_t = xg_sl[j]
                g_t = [act_pool.tile([P, C], bf16, name=f"g{j}_{m}", tag="gtmp", bufs=3)
                       for m in range(MI)]
                for m in range(MI):
                    # wg/wu of each m ride different queues (halves the
                    # per-queue burst for the tile the PE is waiting on)
                    pa, ea, pb, eb = ((wst_s, nc.sync, wst_g, nc.gpsimd) if m % 2 == 0
                                      else (wst_g, nc.gpsimd, wst_s, nc.sync))
                    if j == 0 and m == 0:
                        # cold start: split the first pair's halves across
                        # both queues so the head tile lands ~2x sooner
                        wga = wst_s.tile([P, 8 * P], bf16, name="wg00a", tag="wblk", bufs=4)
                        nc.sync.dma_start(wga[:], wg_h[0][0:P, :8 * P])
                        wgb_ = wst_g.tile([P, 8 * P], bf16, name="wg00b", tag="wblk", bufs=4)
                        nc.gpsimd.dma_start(wgb_[:], wg_h[0][0:P, 8 * P:])
                        wua = wst_s.tile([P, 8 * P], bf16, name="wu00a", tag="wblk", bufs=4)
                        nc.sync.dma_start(wua[:], wu_h[0][0:P, :8 * P])
                        wub_ = wst_g.tile([P, 8 * P], bf16, name="wu00b", tag="wblk", bufs=4)
                        nc.gpsimd.dma_start(wub_[:], wu_h[0][0:P, 8 * P:])
                        wgk = lambda k: (wga if k < 8 else wgb_)[:, (k % 8) * P:(k % 8 + 1) * P]
                        wuk = lambda k: (wua if k < 8 else wub_)[:, (k % 8) * P:(k % 8 + 1) * P]
                    else:
                        wgb = pa.tile([P, KH * P], bf16, name=f"wgb{j}_{m}", tag="wblk", bufs=4)
                        ea.dma_start(wgb[:], wg_h[j][m * P:(m + 1) * P, :])
                        wub = pb.tile([P, KH * P], bf16, name=f"wub{j}_{m}", tag="wblk", bufs=4)
                        eb.dma_start(wub[:], wu_h[j][m * P:(m + 1) * P, :])
                        wgk = lambda k: wgb[:, k * P:(k + 1) * P]
                        wuk = lambda k: wub[:, k * P:(k + 1) * P]
                    for (coff, csz) in _chunks(C, 512):
                        psg = ps_pool.tile([P, csz], f32, name=f"psg{j}_{m}_{coff}", tag="A1", bufs=4)
                        for k in range(KH):
                            nc.tensor.matmul(psg[:], lhsT=wgk(k),
                                             rhs=xg_t[k][:, coff:coff + csz],
                                             start=(k == 0), stop=(k == KH - 1))
                        nc.scalar.activation(g_t[m][:, coff:coff + csz], psg[:], SILU, bias=zbias[:])
                        psu = ps_pool.tile([P, csz], f32, name=f"psu{j}_{m}_{coff}", tag="A1", bufs=4)
                        for k in range(KH):
                            nc.tensor.matmul(psu[:], lhsT=wuk(k),
                                             rhs=xg_t[k][:, coff:coff + csz],
                                             start=(k == 0), stop=(k == KH - 1))
                        # a = silu(g) * u straight out of PSUM, rounded to bf16
                        nc.vector.tensor_mul(out=a_t[j][m][:, coff:coff + csz],
                                             in0=g_t[m][:, coff:coff + csz], in1=psu[:])
                    if hooks is not None:
                        hooks(m)

            def emit_down(j, hooks=None):
                C = CS[j]
                off_j = 0 if j == 0 else C0
                for m in range(MH):
                    if hooks is not None:
                        hooks(m)
                    pool, eng = (dst_s, nc.sync) if m % 2 == 0 else (dst_g, nc.gpsimd)
                    wdb = pool.tile([P, KI * P], bf16, name=f"wdb{j}_{m}", tag="wdb", bufs=3)
                    eng.dma_start(wdb[:], wd_h[j][m * P:(m + 1) * P, :])
                    for (coff, csz) in _chunks(C, 512):
                        psz = ps_pool.tile([P, csz], f32, name=f"psz{j}_{m}_{coff}", tag="A1", bufs=4)
                        for k in range(KI):
                            nc.tensor.matmul(psz[:], lhsT=wdb[:, k * P:(k + 1) * P],
                                             rhs=a_t[j][k][:, coff:coff + csz],
                                             start=(k == 0), stop=(k == KI - 1))
                        zst = stage_pool.tile([P, csz], bf16, name=f"zst{j}_{m}_{coff}", tag="zst", bufs=3)
                        # combine-weight scaling fused into the eviction.
                        # zt rides the scalar queue: a data-dependent trigger
                        # on a weight queue would head-of-line block the
                        # weight stream behind it.
                        nc.vector.tensor_mul(out=zst[:], in0=wb[j][:, coff:coff + csz], in1=psz[:])
                        nc.scalar.dma_start(
                            zt_h[m * P:(m + 1) * P, off_j + coff:off_j + coff + csz], zst[:])

            # full-shared weights: loaded early on the scalar queue (both
            # weight queues are backlogged with routed streams when the
            # shared phases start — sub0 arrived ~10us late on sync)
            sgbt = [None] * NFT
            subt = [None] * NFT

            def load_shared_full(mi):
                sgbt[mi] = sst_pool.tile([P, KH * P], bf16, name=f"sgb{mi}", tag="ssb", bufs=4)
                nc.scalar.dma_start(sgbt[mi][:], swgf_h[mi * P:(mi + 1) * P, :])
                subt[mi] = sst_pool.tile([P, KH * P], bf16, name=f"sub{mi}", tag="ssb", bufs=4)
                nc.scalar.dma_start(subt[mi][:], swuf_h[mi * P:(mi + 1) * P, :])

            def emit_shared_ug_full(mi):
                sgb = sgbt[mi]
                psgs = ps_pool.tile([P, T], f32, name=f"psgs{mi}", tag="B1", bufs=2)
                for k in range(KH):
                    for (noff, nsz) in _chunks(T, 512):
                        nc.tensor.matmul(psgs[:, noff:noff + nsz],
                                         lhsT=sgb[:, k * P:(k + 1) * P],
                                         rhs=xt_t[k][:, noff:noff + nsz],
                                         start=(k == 0), stop=(k == KH - 1))
                nc.scalar.activation(sg_t[mi][:], psgs[:], SILU, bias=zbias[:])
                sub = subt[mi]
                psus = ps_pool.tile([P, T], f32, name=f"psus{mi}", tag="B1", bufs=2)
                for k in range(KH):
                    for (noff, nsz) in _chunks(T, 512):
                        nc.tensor.matmul(psus[:, noff:noff + nsz],
                                         lhsT=sub[:, k * P:(k + 1) * P],
                                         rhs=xt_t[k][:, noff:noff + nsz],
                                         start=(k == 0), stop=(k == KH - 1))
                nc.vector.tensor_mul(out=as_t[mi][:], in0=sg_t[mi][:], in1=psus[:])

            def emit_shared_ug_unit(i):
                w0 = i * UW
                sgu = sst_pool.tile([P, KH * P], bf16, name=f"sgu{i}", tag="ssb", bufs=4)
                nc.gpsimd.dma_start(sgu[:], swgu_h[i * P:(i + 1) * P, :])
                psg = ps_pool.tile([P, UW], f32, name=f"psgu{i}", tag="A1", bufs=4)
                for k in range(KH):
                    nc.tensor.matmul(psg[:], lhsT=sgu[:, k * P:(k + 1) * P],
                                     rhs=xt_t[k][:, w0:w0 + UW],
                                     start=(k == 0), stop=(k == KH - 1))
                sgt = small_pool.tile([P, UW], bf16, name=f"sgut{i}", tag="sgut")
                nc.scalar.activation(sgt[:], psg[:], SILU, bias=zbias[:])
                suu = sst_pool.tile([P, KH * P], bf16, name=f"suu{i}", tag="ssb", bufs=4)
                nc.sync.dma_start(suu[:], swuu_h[i * P:(i + 1) * P, :])
                psu = ps_pool.tile([P, UW], f32, name=f"psuu{i}", tag="A1", bufs=4)
                for k in range(KH):
                    nc.tensor.matmul(psu[:], lhsT=suu[:, k * P:(k + 1) * P],
                                     rhs=xt_t[k][:, w0:w0 + UW],
                                     start=(k == 0), stop=(k == KH - 1))
                nc.vector.tensor_mul(out=asu_t[i][:], in0=sgt[:], in1=psu[:])

            def emit_shared_down():
                for m in range(MH):
                    # dedicated pool + alternating queues: sharing the ssb
                    # pool made sdb loads wait on unit-weight slot frees
                    pool, eng = (dst_s, nc.sync) if m % 2 == 0 else (dst_g, nc.gpsimd)
                    sdb = pool.tile([P, NFT * P], bf16, name=f"sdb{m}", tag="sdb", bufs=6)
                    eng.dma_start(sdb[:], swdf_h[m * P:(m + 1) * P, :])
                    psys = ps_pool.tile([P, T], f32, name=f"psys{m}", tag="B1", bufs=2)
                    for r in range(4):
                        r0 = r * UW
                        nc.tensor.matmul(psys[:, r0:r0 + UW], lhsT=sdb[:, 0:P],
                                         rhs=as_t[0][:, r0:r0 + UW],
                                         start=True, stop=False)
                        nc.tensor.matmul(psys[:, r0:r0 + UW], lhsT=sdb[:, P:2 * P],
                                         rhs=as_t[1][:, r0:r0 + UW],
                                         start=False, stop=(r == 3))
                        if r < 3:
                            nc.tensor.matmul(psys[:, r0:r0 + UW],
                                             lhsT=sdub[r][:, m * P:(m + 1) * P],
                                             rhs=asu_t[r][:],
                                             start=False, stop=True)
                    sstg = stage_pool.tile([P, T], bf16, name=f"sstg{m}", tag="sstage", bufs=3)
                    # evict halves on vector+scalar in parallel: a single
                    # engine's f32->bf16 cast (~1.2us per [128,1024]) paced
                    # the whole tail through the 2-slot PSUM pool
                    nc.vector.tensor_copy(sstg[:, 0:T // 2], psys[:, 0:T // 2])
                    nc.scalar.copy(sstg[:, T // 2:T], psys[:, T // 2:T])
                    # st writes alternate queues (a single queue can't keep
                    # up with the tail eviction rate)
                    eng = nc.scalar if m % 2 == 0 else nc.gpsimd
                    eng.dma_start(st_h[m * P:(m + 1) * P, :], sstg[:])

            def up0_hooks(m):
                defer_loads(m)
                # gate logits after the last silu: a mid-phase EXP would
                # thrash the scalar engine's activation tables (1.3us per
                # reload) right where the a_t chain is critical
                if m == MI - 1:
                    load_shared_full(0)
                    emit_gate_logits(0)
                    emit_gate_logits(1)

            def down0_hooks(m):
                if m == 8:
                    load_shared_full(1)

            emit_upgate(0, hooks=up0_hooks)
            emit_gate_broadcast(0)
            emit_gate_broadcast(1)
            emit_down(0, hooks=down0_hooks)
            emit_shared_ug_full(0)
            emit_upgate(1)
            emit_down(1)
            emit_shared_ug_full(1)
            emit_shared_ug_unit(0)
            emit_shared_ug_unit(1)
            emit_shared_ug_unit(2)
            emit_shared_down()

    nc.compile()
    return nc


def _get_nc(C0, C1):
    key = (C0, C1)
    if key not in _NC_CACHE:
        _NC_CACHE[key] = _build(C0, C1)
    return _NC_CACHE[key]


def kernel(**inputs):
    global LAST_RESULTS
    from concourse.bass_utils import run_bass_kernel_spmd

    hs = np.asarray(inputs["hidden_states"], dtype=np.float32)
    gate_w = np.asarray(inputs["gate_w"], dtype=np.float32)
    w_gate = np.asarray(inputs["w_gate"], dtype=np.float32)
    w_up = np.asarray(inputs["w_up"], dtype=np.float32)
    w_down = np.asarray(inputs["w_down"], dtype=np.float32)
    sw_gate = np.asarray(inputs["sw_gate"], dtype=np.float32)
    sw_up = np.asarray(inputs["sw_up"], dtype=np.float32)
    sw_down = np.asarray(inputs["sw_down"], dtype=np.float32)

    orig_shape = hs.shape
    x = hs.reshape(-1, H)
    assert x.shape[0] == T

    # ---- host: discrete routing only (top-4 selection + dispatch tables) ----
    logits = x @ gate_w.T
    smax = logits.max(axis=-1, keepdims=True)
    sc = np.exp(logits - smax)
    sc /= sc.sum(axis=-1, keepdims=True)
    order = np.argsort(-sc, axis=-1, kind="stable")[:, :TOPK]
    mask = np.zeros((T, E), dtype=np.float32)
    mask[np.arange(T)[:, None], order] = 1.0
    tok_lists = [np.nonzero(mask[:, e])[0].astype(np.int64) for e in range(E)]
    counts = np.array([len(tk) for tk in tok_lists])

    # slot 0 = the 8 largest-load experts, slot 1 = the 8 smallest
    rank = np.argsort(-counts, kind="stable")
    slot_experts = [rank[:NCORES], rank[NCORES:][::-1]]  # pair big with small
    C0 = int(counts[slot_experts[0]].max())
    C1 = int(counts[slot_experts[1]].max())

    nc = _get_nc(C0, C1)
    CS = [C0, C1]
    NCH = [math.ceil(c / P) for c in CS]

    xb = x.astype(BF16)
    xTb = np.ascontiguousarray(xb.T)                     # [H, T] bf16
    gate_wT = gate_w.T.astype(BF16)                      # [H, E] bf16

    swg_b = sw_gate.astype(BF16)
    swu_b = sw_up.astype(BF16)
    swd_b = sw_down.astype(BF16)
    wg_b = w_gate.astype(BF16)
    wu_b = w_up.astype(BF16)
    wd_b = w_down.astype(BF16)

    in_maps = []
    core_experts = []
    rhos = []
    for c in range(NCORES):
        es = [int(slot_experts[j][c]) for j in range(EPC)]
        core_experts.append(es)
        rho = UW * ((NUT * c) % 4)
        rhos.append(rho)
        # token rotation for the split shared tiles (windows land at fixed
        # rotated offsets 0/256/512 on every core)
        xrotT = np.roll(xTb, -rho, axis=1)               # [H, T] rotated cols
        xtb = np.ascontiguousarray(
            xrotT.reshape(KH, P, T).transpose(1, 0, 2).reshape(P, KH * T))
        # full shared tiles: intermediate cols [256c, 256c+256)
        fcols = slice(NFT * P * c, NFT * P * (c + 1))
        # split units
        ucols = []
        for i in range(NUT):
            u = NUT * c + i
            t0 = 2 * NCORES * P + (u // 4) * P           # 2048 + 128*(u//4)
            ucols.append(np.arange(t0, t0 + P))
        ucat = np.concatenate(ucols)
        im = {
            "xt": xtb,
            "swgf": _pack_st(swg_b[:, fcols], KH, NFT),
            "swuf": _pack_st(swu_b[:, fcols], KH, NFT),
            "swdf": _pack_st(swd_b[fcols, :], NFT, MH),
            "swgu": np.concatenate(
                [_pack_st(swg_b[:, uc], KH, 1) for uc in ucols], axis=0),
            "swuu": np.concatenate(
                [_pack_st(swu_b[:, uc], KH, 1) for uc in ucols], axis=0),
            "sdu": np.ascontiguousarray(swd_b[ucat, :]),
        }
        for j, e in enumerate(es):
            tk = tok_lists[e]
            n = len(tk)
            C = CS[j]
            # gathered activations, tile-major: xg[p, k*C + s] = x[tok_s, k*P + p]
            xg = np.zeros((P, KH * C), dtype=BF16)
            g = xTb[:, tk].reshape(KH, P, n).transpose(1, 0, 2)  # [P, KH, n]
            xg.reshape(P, KH, C)[:, :, :n] = g
            im[f"xg{j}"] = xg
            # gate weights + mask rows share a per-slot permutation with the
            # slot's own expert in column 0 (softmax sums are perm-invariant)
            perm = [e] + [q for q in range(E) if q != e]
            gwtp = gate_wT[:, perm]                              # [H, E]
            im[f"gwt{j}"] = np.ascontiguousarray(
                gwtp.reshape(KH, P, E).transpose(1, 0, 2).reshape(P, KH * E))
            mkc = np.zeros((P, NCH[j] * E), dtype=np.float32)
            mrows = mask[tk][:, perm]                            # [n, E]
            for ch in range(NCH[j]):
                lo = ch * P
                sz = min(P, n - lo)
                if sz > 0:
                    mkc[:sz, ch * E:(ch + 1) * E] = mrows[lo:lo + sz]
            im[f"mk{j}"] = mkc
            im[f"wg{j}"] = _pack_st(wg_b[e], KH, MI)
            im[f"wu{j}"] = _pack_st(wu_b[e], KH, MI)
            im[f"wd{j}"] = _pack_st(wd_b[e], KI, MH)
        in_maps.append(im)

    trace = bool(int(os.environ.get("BASSMOE_TRACE", "0")))
    kwargs = {}
    if trace:
        kwargs = dict(trace=True, tmpdir=os.environ.get("BASSMOE_TRACE_DIR") or None)
        tcores = os.environ.get("BASSMOE_TRACE_CORES")
        if tcores:
            kwargs["trace_cores"] = [int(x) for x in tcores.split(",")]
            kwargs["stitch_traces"] = False
    res = run_bass_kernel_spmd(nc, in_maps, core_ids=list(range(NCORES)), **kwargs)
    LAST_RESULTS = res

    # ---- host: unshard (scatter-add compact expert outputs + sum partials) ----
    y = np.zeros((T, H), dtype=np.float64)
    st_sum = np.zeros((H, T), dtype=np.float64)
    for c in range(NCORES):
        r = res.results[c]
        # st is in this core's rotated token order; unrotate
        st_sum += np.roll(np.asarray(r["st"], dtype=np.float32), rhos[c], axis=1)
        zt = np.asarray(r["zt"], dtype=np.float32)
        for j in range(EPC):
            e = core_experts[c][j]
            tk = tok_lists[e]
            off = 0 if j == 0 else C0
            y[tk] += zt[:, off:off + len(tk)].T
    y += st_sum.T
    return y.astype(np.float32).reshape(orig_shape)
